# revision 3
# baseline (speedup 1.0000x reference)
"""Trainium2 Bass kernel for nn_BaselineMNISTClassifier (vq_codebook).

reference:
    x = samples - 0.5                        # [B, F]
    hv = einsum('bf,df->bd', x, bhv)         # [B, D]
    e = (hv > 0)                             # binary
    ham[b, c] = sum_d |e - centroids[c, d]|  # [B, C]
    return -ham

Identity used on device: with e' = (hv > 0) - 0.5 in {-1/2, +1/2} and
cmod = 1 - 2c in {-1, +1}:  |e - c| = e'* cmod + 1/2, so
    ham[b, c] = sum_d e'[b, d] * cmod[c, d] + D/2
which turns the broadcast Hamming into a second (tiny) matmul.

Sharding: D axis (10000) split across 8 cores, 1250 (padded to 1280) per
core. Each core computes full-batch partial hamming [C, B]; partials sum
on the host (padded dims contribute exactly 0: centroid pad value 0.5
makes cmod = 0 there).

Encode matmul runs in float32r (full PE rate at N=512, ~tf32 precision).
Both operands are host-transposed so the contraction dim F sits on SBUF
partitions. Second matmul runs in bf16 (e', cmod are exact in bf16).
"""

import sys

sys.path.insert(0, "/opt/trn_rl_repo")

import numpy as np

import concourse.bacc as bacc
import concourse.bass as bass
import concourse.mybir as mybir
import concourse.tile as tile
from concourse.bass_utils import run_bass_kernel_spmd

B = 4096
F = 784
D = 10000
C = 10
NCORES = 8
DREAL = D // NCORES          # 1250 real dims per core
DP = 1280                    # padded to 10 d-tiles of 128
ND = DP // 128               # 10
NB = B // 512                # 8 b-blocks of 512
FT = [(i * 128, min(128, F - i * 128)) for i in range((F + 127) // 128)]
NF = len(FT)                 # 7 (6x128 + 16)

F32 = mybir.dt.float32
F32R = mybir.dt.float32r
BF16 = mybir.dt.bfloat16
OP = mybir.AluOpType

_NC_CACHE = {}


def _build_nc():
    if "nc" in _NC_CACHE:
        return _NC_CACHE["nc"]
    nc = bacc.Bacc("TRN2", debug=False, target_bir_lowering=False)
    xT = nc.dram_tensor("xT", [F, B], F32R, kind="ExternalInput")
    wT = nc.dram_tensor("wT", [F, DP], F32R, kind="ExternalInput")
    cT = nc.dram_tensor("cT", [DP, C], F32, kind="ExternalInput")
    out = nc.dram_tensor("out", [C, B], F32, kind="ExternalOutput")

    with tile.TileContext(nc) as tc:
        with (
            tc.tile_pool(name="xp", bufs=NB * NF) as xpool,
            tc.tile_pool(name="wp", bufs=ND * NF) as wpool,
            tc.tile_pool(name="cp", bufs=ND) as cpool,
            tc.tile_pool(name="cmp", bufs=ND) as cmpool,
            tc.tile_pool(name="ep", bufs=4) as epool,
            tc.tile_pool(name="op", bufs=NB) as opool,
            tc.tile_pool(name="pse", bufs=4, space="PSUM") as psepool,
            tc.tile_pool(name="ps2", bufs=4, space="PSUM") as ps2pool,
        ):
            # Centroid prep: cmod = 1 - 2c (bf16); pad rows were set to
            # 0.5 on the host so cmod = 0 there.
            cmods = []
            for di in range(ND):
                ct = cpool.tile([128, C], F32)
                nc.sync.dma_start(ct[:], cT[di * 128:(di + 1) * 128, :])
                cm = cmpool.tile([128, C], BF16)
                nc.vector.tensor_scalar(cm[:], ct[:], -2.0, 1.0,
                                        op0=OP.mult, op1=OP.add)
                cmods.append(cm)

            # Input loads. All tiles single-assignment (loaded once, no
            # slot reuse) so input DMAs never carry data-dependency waits.
            # Emission order interleaves the first b-block / d-tile so the
            # PE can start early.
            xts = {}
            wts = {}

            def load_x(bb):
                for fi, (f0, fl) in enumerate(FT):
                    xt = xpool.tile([fl, 512], F32R, name=f"xt_{bb}_{fi}", tag="xt")
                    nc.sync.dma_start(
                        xt[:], xT[f0:f0 + fl, bb * 512:(bb + 1) * 512])
                    # center (x - 0.5) in place on DVE
                    nc.vector.tensor_scalar_add(xt[:], xt[:], -0.5)
                    xts[bb, fi] = xt

            def load_w(di):
                for fi, (f0, fl) in enumerate(FT):
                    wt = wpool.tile([fl, 128], F32R, name=f"wt_{di}_{fi}", tag="wt")
                    nc.sync.dma_start(
                        wt[:], wT[f0:f0 + fl, di * 128:(di + 1) * 128])
                    wts[di, fi] = wt

            for i in range(ND):
                if i < NB:
                    load_x(i)
                load_w(i)

            # Main compute. Two b-groups of 4 blocks each: 4 PSUM banks
            # accumulate the encode matmul, 4 hold the hamming partials.
            # The hamming matmul for a block is emitted one step late so
            # the PE never stalls on the DVE binarize.
            for bg in range(2):
                bbs = range(bg * 4, bg * 4 + 4)
                psum2 = {}
                for bb in bbs:
                    psum2[bb] = ps2pool.tile([C, 512], F32, name=f"ps2_{bb}",
                                             tag="ps2")
                pending = None
                for di in range(ND):
                    for bb in bbs:
                        pse = psepool.tile([128, 512], F32)
                        for fi in range(NF):
                            nc.tensor.matmul(pse[:], wts[di, fi][:],
                                             xts[bb, fi][:],
                                             start=(fi == 0),
                                             stop=(fi == NF - 1))
                        # e' = (hv > 0) - 0.5 in {-1/2, +1/2}
                        et = epool.tile([128, 512], BF16)
                        nc.vector.tensor_scalar(et[:], pse[:], 0.0, 0.5,
                                                op0=OP.is_gt,
                                                op1=OP.subtract)
                        if pending is not None:
                            pdi, pbb, pet = pending
                            nc.tensor.matmul(psum2[pbb][:], cmods[pdi][:],
                                             pet[:], start=(pdi == 0),
                                             stop=(pdi == ND - 1))
                        pending = (di, bb, et)
                pdi, pbb, pet = pending
                nc.tensor.matmul(psum2[pbb][:], cmods[pdi][:], pet[:],
                                 start=(pdi == 0), stop=(pdi == ND - 1))
                # out = -(psum2 + DREAL/2)
                for bb in bbs:
                    ot = opool.tile([C, 512], F32)
                    nc.vector.tensor_scalar(ot[:], psum2[bb][:],
                                            float(DREAL) / 2.0, -1.0,
                                            op0=OP.add, op1=OP.mult)
                    nc.gpsimd.dma_start(out[:, bb * 512:(bb + 1) * 512],
                                        ot[:])
    nc.compile()
    _NC_CACHE["nc"] = nc
    return nc


def _prep_in_maps(samples, bhv_matrix, centroids):
    samples = np.ascontiguousarray(samples, dtype=np.float32)
    bhv_matrix = np.ascontiguousarray(bhv_matrix, dtype=np.float32)
    centroids = np.ascontiguousarray(centroids, dtype=np.float32)
    xT = np.ascontiguousarray(samples.T)  # [F, B]
    in_maps = []
    for k in range(NCORES):
        lo, hi = k * DREAL, (k + 1) * DREAL
        wTk = np.zeros((F, DP), dtype=np.float32)
        wTk[:, :DREAL] = bhv_matrix[lo:hi, :].T
        cTk = np.full((DP, C), 0.5, dtype=np.float32)
        cTk[:DREAL, :] = centroids[:, lo:hi].T
        in_maps.append({"xT": xT, "wT": wTk, "cT": cTk})
    return in_maps


def _run(samples, bhv_matrix, centroids, **spmd_kwargs):
    nc = _build_nc()
    in_maps = _prep_in_maps(samples, bhv_matrix, centroids)
    res = run_bass_kernel_spmd(nc, in_maps, core_ids=list(range(NCORES)),
                               **spmd_kwargs)
    acc = np.zeros((C, B), dtype=np.float32)
    for r in res.results:
        acc += r["out"]
    return np.ascontiguousarray(acc.T), res


def kernel(samples, bhv_matrix, centroids):
    out, _ = _run(samples, bhv_matrix, centroids)
    return out


# revision 6
# speedup vs baseline: 1.1367x; 1.1367x over previous
"""Trainium2 Bass kernel for nn_BaselineMNISTClassifier (vq_codebook).

reference:
    x = samples - 0.5                        # [B, F]
    hv = einsum('bf,df->bd', x, bhv)         # [B, D]
    e = (hv > 0)                             # binary
    ham[b, c] = sum_d |e - centroids[c, d]|  # [B, C]
    return -ham

Identity used on device: with e' = (hv > 0) - 0.5 in {-1/2, +1/2} and
cmod = 1 - 2c in {-1, +1}:  |e - c| = e'* cmod + 1/2, so
    ham[b, c] = sum_d e'[b, d] * cmod[c, d] + D/2
which turns the broadcast Hamming into a second (tiny) matmul.

Sharding: D axis (10000) split across 8 cores, 1250 (padded to 1280) per
core. Each core computes full-batch partial hamming [C, B]; partials sum
on the host (padded dims contribute exactly 0: centroid pad value 0.5
makes cmod = 0 there).

Encode matmul runs in float32r (full PE rate at N=512, ~tf32 precision).
Both operands are host-transposed so the contraction dim F sits on SBUF
partitions. Second matmul runs in bf16 (e', cmod are exact in bf16).
"""

import sys

sys.path.insert(0, "/opt/trn_rl_repo")

import numpy as np

import concourse.bacc as bacc
import concourse.bass as bass
import concourse.mybir as mybir
import concourse.tile as tile
from concourse.bass_utils import run_bass_kernel_spmd

B = 4096
F = 784
D = 10000
C = 10
NCORES = 8
DREAL = D // NCORES          # 1250 real dims per core
DP = 1280                    # padded to 10 d-tiles of 128
ND = DP // 128               # 10
NB = B // 512                # 8 b-blocks of 512
FT = [(i * 128, min(128, F - i * 128)) for i in range((F + 127) // 128)]
NF = len(FT)                 # 7 (6x128 + 16)

F32 = mybir.dt.float32
F32R = mybir.dt.float32r
BF16 = mybir.dt.bfloat16
OP = mybir.AluOpType

_NC_CACHE = {}


def _build_nc():
    if "nc" in _NC_CACHE:
        return _NC_CACHE["nc"]
    nc = bacc.Bacc("TRN2", debug=False, target_bir_lowering=False)
    xT = nc.dram_tensor("xT", [F, B], F32R, kind="ExternalInput")
    wT = nc.dram_tensor("wT", [F, DP], F32R, kind="ExternalInput")
    cT = nc.dram_tensor("cT", [DP, C], F32, kind="ExternalInput")
    out = nc.dram_tensor("out", [C, B], F32, kind="ExternalOutput")

    with tile.TileContext(nc) as tc:
        with (
            tc.tile_pool(name="xp", bufs=NB * NF) as xpool,
            tc.tile_pool(name="wp", bufs=ND * NF) as wpool,
            tc.tile_pool(name="cp", bufs=ND) as cpool,
            tc.tile_pool(name="cmp", bufs=ND) as cmpool,
            tc.tile_pool(name="ep", bufs=8) as epool,
            tc.tile_pool(name="op", bufs=NB) as opool,
            tc.tile_pool(name="pse", bufs=4, space="PSUM") as psepool,
            tc.tile_pool(name="ps2", bufs=4, space="PSUM") as ps2pool,
        ):
            # Centroid prep: cmod = 1 - 2c (bf16); pad rows were set to
            # 0.5 on the host so cmod = 0 there.
            cmods = []
            for di in range(ND):
                ct = cpool.tile([128, C], F32)
                nc.sync.dma_start(ct[:], cT[di * 128:(di + 1) * 128, :])
                cm = cmpool.tile([128, C], BF16)
                nc.vector.tensor_scalar(cm[:], ct[:], -2.0, 1.0,
                                        op0=OP.mult, op1=OP.add)
                cmods.append(cm)

            # Input loads. All tiles single-assignment (loaded once, no
            # slot reuse) so input DMAs never carry data-dependency waits.
            # Emission order interleaves the first b-block / d-tile so the
            # PE can start early.
            xts = {}
            wts = {}

            def load_x(bb, fi):
                f0, fl = FT[fi]
                xt = xpool.tile([fl, 512], F32R, name=f"xt_{bb}_{fi}",
                                tag="xt")
                nc.sync.dma_start(
                    xt[:], xT[f0:f0 + fl, bb * 512:(bb + 1) * 512])
                # center (x - 0.5) in place on DVE
                nc.vector.tensor_scalar_add(xt[:], xt[:], -0.5)
                xts[bb, fi] = xt

            def load_w(di, fi):
                f0, fl = FT[fi]
                wt = wpool.tile([fl, 128], F32R, name=f"wt_{di}_{fi}",
                                tag="wt")
                nc.sync.dma_start(
                    wt[:], wT[f0:f0 + fl, di * 128:(di + 1) * 128])
                wts[di, fi] = wt

            for i in range(ND):
                for fi in range(NF):
                    if i < NB:
                        load_x(i, fi)
                    load_w(i, fi)

            # Main compute. Two b-groups of 4 blocks each: 4 PSUM banks
            # accumulate the encode matmul, 4 hold the hamming partials.
            # The hamming matmul for a block is emitted one step late so
            # the PE never stalls on the DVE binarize.
            for bg in range(2):
                bbs = range(bg * 4, bg * 4 + 4)
                psum2 = {}
                for bb in bbs:
                    psum2[bb] = ps2pool.tile([C, 512], F32, name=f"ps2_{bb}",
                                             tag="ps2")
                pending = []
                for di in range(ND):
                    # fi-outer / bb-inner: 4 consecutive matmuls share the
                    # same stationary weights (one LDWEIGHTS with ldw-opt).
                    pses = {}
                    for bb in bbs:
                        pses[bb] = psepool.tile([128, 512], F32,
                                                name=f"pse_{di % 2}_{bb}",
                                                tag="pse")
                    for fi in range(NF):
                        for bb in bbs:
                            nc.tensor.matmul(pses[bb][:], wts[di, fi][:],
                                             xts[bb, fi][:],
                                             start=(fi == 0),
                                             stop=(fi == NF - 1))
                    ets = {}
                    for bb in bbs:
                        # e' = (hv > 0) - 0.5 in {-1/2, +1/2}
                        et = epool.tile([128, 512], BF16,
                                        name=f"et_{di % 2}_{bb}", tag="et")
                        nc.vector.tensor_scalar(et[:], pses[bb][:], 0.0, 0.5,
                                                op0=OP.is_gt,
                                                op1=OP.subtract)
                        ets[bb] = et
                    # hamming matmuls for the previous d-tile (one step
                    # late so the PE never stalls on the DVE binarize)
                    for pdi, pbb, pet in pending:
                        nc.tensor.matmul(psum2[pbb][:], cmods[pdi][:],
                                         pet[:], start=(pdi == 0),
                                         stop=(pdi == ND - 1))
                    pending = [(di, bb, ets[bb]) for bb in bbs]
                for pdi, pbb, pet in pending:
                    nc.tensor.matmul(psum2[pbb][:], cmods[pdi][:], pet[:],
                                     start=(pdi == 0), stop=(pdi == ND - 1))
                # out = -(psum2 + DREAL/2)
                for bb in bbs:
                    ot = opool.tile([C, 512], F32)
                    nc.vector.tensor_scalar(ot[:], psum2[bb][:],
                                            float(DREAL) / 2.0, -1.0,
                                            op0=OP.add, op1=OP.mult)
                    nc.gpsimd.dma_start(out[:, bb * 512:(bb + 1) * 512],
                                        ot[:])
    nc.compile()
    _NC_CACHE["nc"] = nc
    return nc


def _prep_in_maps(samples, bhv_matrix, centroids):
    samples = np.ascontiguousarray(samples, dtype=np.float32)
    bhv_matrix = np.ascontiguousarray(bhv_matrix, dtype=np.float32)
    centroids = np.ascontiguousarray(centroids, dtype=np.float32)
    xT = np.ascontiguousarray(samples.T)  # [F, B]
    in_maps = []
    for k in range(NCORES):
        lo, hi = k * DREAL, (k + 1) * DREAL
        wTk = np.zeros((F, DP), dtype=np.float32)
        wTk[:, :DREAL] = bhv_matrix[lo:hi, :].T
        cTk = np.full((DP, C), 0.5, dtype=np.float32)
        cTk[:DREAL, :] = centroids[:, lo:hi].T
        in_maps.append({"xT": xT, "wT": wTk, "cT": cTk})
    return in_maps


def _run(samples, bhv_matrix, centroids, **spmd_kwargs):
    nc = _build_nc()
    in_maps = _prep_in_maps(samples, bhv_matrix, centroids)
    res = run_bass_kernel_spmd(nc, in_maps, core_ids=list(range(NCORES)),
                               **spmd_kwargs)
    acc = np.zeros((C, B), dtype=np.float32)
    for r in res.results:
        acc += r["out"]
    return np.ascontiguousarray(acc.T), res


def kernel(samples, bhv_matrix, centroids):
    out, _ = _run(samples, bhv_matrix, centroids)
    return out


# revision 7
# speedup vs baseline: 1.2227x; 1.0756x over previous
"""Trainium2 Bass kernel for nn_BaselineMNISTClassifier (vq_codebook).

reference:
    x = samples - 0.5                        # [B, F]
    hv = einsum('bf,df->bd', x, bhv)         # [B, D]
    e = (hv > 0)                             # binary
    ham[b, c] = sum_d |e - centroids[c, d]|  # [B, C]
    return -ham

Identity used on device: with e' = (hv > 0) - 0.5 in {-1/2, +1/2} and
cmod = 1 - 2c in {-1, +1}:  |e - c| = e'* cmod + 1/2, so
    ham[b, c] = sum_d e'[b, d] * cmod[c, d] + D/2
which turns the broadcast Hamming into a second (tiny) matmul.

Sharding: D axis (10000) split across 8 cores, 1250 (padded to 1280) per
core. Each core computes full-batch partial hamming [C, B]; partials sum
on the host (padded dims contribute exactly 0: centroid pad value 0.5
makes cmod = 0 there).

Encode matmul runs in float32r (full PE rate at N=512, ~tf32 precision).
Both operands are host-transposed so the contraction dim F sits on SBUF
partitions. Second matmul runs in bf16 (e', cmod are exact in bf16).

Perf structure (per core, one NeuronCore):
  - warmup matmuls on dummy data release the PE HAM clock gate while
    inputs stream in
  - input DMA triggers split across SP (x) and GpSimd (w, c) queues
  - fi-outer/bb-inner matmul order: 4 consecutive matmuls share weights
  - hamming matmuls emitted one d-tile late so PE never waits on DVE
  - epilogue on the Scalar engine, output DMA per b-block as soon as its
    hamming accumulation finishes
"""

import sys

sys.path.insert(0, "/opt/trn_rl_repo")

import numpy as np

import concourse.bacc as bacc
import concourse.bass as bass
import concourse.mybir as mybir
import concourse.tile as tile
from concourse.bass_utils import run_bass_kernel_spmd

B = 4096
F = 784
D = 10000
C = 10
NCORES = 8
DREAL = D // NCORES          # 1250 real dims per core
DP = 1280                    # padded to 10 d-tiles of 128
ND = DP // 128               # 10
NB = B // 512                # 8 b-blocks of 512
FT = [(i * 128, min(128, F - i * 128)) for i in range((F + 127) // 128)]
NF = len(FT)                 # 7 (6x128 + 16)
NWARM = 28                   # PE warmup matmuls

F32 = mybir.dt.float32
F32R = mybir.dt.float32r
BF16 = mybir.dt.bfloat16
OP = mybir.AluOpType
AF = mybir.ActivationFunctionType

_NC_CACHE = {}


def _build_nc():
    if "nc" in _NC_CACHE:
        return _NC_CACHE["nc"]
    nc = bacc.Bacc("TRN2", debug=False, target_bir_lowering=False)
    xT = nc.dram_tensor("xT", [F, B], F32R, kind="ExternalInput")
    wT = nc.dram_tensor("wT", [F, DP], F32R, kind="ExternalInput")
    cT = nc.dram_tensor("cT", [DP, C], F32, kind="ExternalInput")
    out = nc.dram_tensor("out", [C, B], F32, kind="ExternalOutput")

    with tile.TileContext(nc) as tc:
        with (
            tc.tile_pool(name="dum", bufs=2) as dumpool,
            tc.tile_pool(name="xp", bufs=NB // 2 * NF) as xpool,
            tc.tile_pool(name="wp", bufs=(ND + 1) // 2 * NF) as wpool,
            tc.tile_pool(name="cp", bufs=1) as cpool,
            tc.tile_pool(name="cmp", bufs=1) as cmpool,
            tc.tile_pool(name="ep", bufs=8) as epool,
            tc.tile_pool(name="op", bufs=NB) as opool,
            tc.tile_pool(name="pse", bufs=4, space="PSUM") as psepool,
            tc.tile_pool(name="ps2", bufs=4, space="PSUM") as ps2pool,
        ):
            # --- PE warmup: release the HAM clock gate while inputs load.
            wdum = dumpool.tile([128, 128], BF16)
            nc.gpsimd.memset(wdum[:], 1.0)
            xdum = dumpool.tile([128, 512], BF16)
            nc.gpsimd.memset(xdum[:], 1.0)
            psdum = psepool.tile([128, 512], F32, name="psdum", tag="pse")
            for i in range(NWARM):
                nc.tensor.matmul(psdum[:], wdum[:], xdum[:],
                                 start=(i == 0), stop=(i == NWARM - 1))

            # --- centroid prep: one DMA for all 10 d-tiles, then
            # cmod = 1 - 2c (bf16). Pad rows are 0.5 -> cmod = 0.
            ct = cpool.tile([128, ND * C], F32)
            nc.gpsimd.dma_start(
                ct[:], cT.rearrange("(a p) c -> p a c", p=128))
            cmod = cmpool.tile([128, ND * C], BF16)
            nc.scalar.activation(cmod[:], ct[:], AF.Copy, bias=1.0,
                                 scale=-2.0)
            cmods = [cmod[:, di * C:(di + 1) * C] for di in range(ND)]

            # --- input loads; tiles are single-assignment (no slot reuse)
            # so input DMAs never carry data-dependency waits. x triggers
            # on SP, w triggers on GpSimd (parallel issue). x tiles span
            # two b-blocks, w tiles two d-tiles.
            xts = {}
            wts = {}

            def load_x(bp, fi):   # bp = b-block pair index (0..3)
                f0, fl = FT[fi]
                xt = xpool.tile([fl, 1024], F32R, name=f"xt_{bp}_{fi}",
                                tag="xt")
                nc.sync.dma_start(
                    xt[:], xT[f0:f0 + fl, bp * 1024:(bp + 1) * 1024])
                nc.vector.tensor_scalar_add(xt[:], xt[:], -0.5)
                xts[bp, fi] = xt

            def load_w(dp, fi):   # dp = d-tile pair index (0..4)
                f0, fl = FT[fi]
                wid = min(256, DP - dp * 256)
                wt = wpool.tile([fl, wid], F32R, name=f"wt_{dp}_{fi}",
                                tag="wt")
                nc.gpsimd.dma_start(
                    wt[:], wT[f0:f0 + fl, dp * 256:dp * 256 + wid])
                wts[dp, fi] = wt

            for i in range(5):
                for fi in range(NF):
                    if i < 4:
                        load_x(i, fi)
                    load_w(i, fi)

            def xop(bb, fi):
                return xts[bb // 2, fi][:, (bb % 2) * 512:(bb % 2 + 1) * 512]

            def wop(di, fi):
                return wts[di // 2, fi][:, (di % 2) * 128:(di % 2 + 1) * 128]

            # --- main compute: two b-groups of 4 blocks (4 PSUM banks for
            # encode accumulation + 4 for hamming partials).
            for bg in range(2):
                bbs = list(range(bg * 4, bg * 4 + 4))
                psum2 = {}
                for bb in bbs:
                    psum2[bb] = ps2pool.tile([C, 512], F32,
                                             name=f"ps2_{bb}", tag="ps2")
                pending = []
                for di in range(ND):
                    pses = {}
                    for bb in bbs:
                        pses[bb] = psepool.tile([128, 512], F32,
                                                name=f"pse_{di % 2}_{bb}",
                                                tag="pse")
                    for fi in range(NF):
                        for bb in bbs:
                            nc.tensor.matmul(pses[bb][:], wop(di, fi),
                                             xop(bb, fi),
                                             start=(fi == 0),
                                             stop=(fi == NF - 1))
                    ets = {}
                    for bb in bbs:
                        # e' = (hv > 0) - 0.5 in {-1/2, +1/2}
                        et = epool.tile([128, 512], BF16,
                                        name=f"et_{di % 2}_{bb}", tag="et")
                        nc.vector.tensor_scalar(et[:], pses[bb][:], 0.0,
                                                0.5, op0=OP.is_gt,
                                                op1=OP.subtract)
                        ets[bb] = et
                    # hamming matmuls one d-tile late (PE never stalls on
                    # the DVE binarize); last d-tile issues immediately.
                    for pdi, pbb, pet in pending:
                        nc.tensor.matmul(psum2[pbb][:], cmods[pdi],
                                         pet[:], start=(pdi == 0),
                                         stop=(pdi == ND - 1))
                    pending = [(di, bb, ets[bb]) for bb in bbs]
                for pdi, pbb, pet in pending:
                    nc.tensor.matmul(psum2[pbb][:], cmods[pdi], pet[:],
                                     start=(pdi == 0), stop=(pdi == ND - 1))
                    # out = -(psum2 + DREAL/2), on the (idle) Scalar engine
                    ot = opool.tile([C, 512], F32, name=f"ot_{pbb}",
                                    tag="ot")
                    nc.scalar.activation(ot[:], psum2[pbb][:], AF.Copy,
                                         bias=-float(DREAL) / 2.0,
                                         scale=-1.0)
                    nc.gpsimd.dma_start(
                        out[:, pbb * 512:(pbb + 1) * 512], ot[:])
    nc.compile()
    _NC_CACHE["nc"] = nc
    return nc


def _prep_in_maps(samples, bhv_matrix, centroids):
    samples = np.ascontiguousarray(samples, dtype=np.float32)
    bhv_matrix = np.ascontiguousarray(bhv_matrix, dtype=np.float32)
    centroids = np.ascontiguousarray(centroids, dtype=np.float32)
    xT = np.ascontiguousarray(samples.T)  # [F, B]
    in_maps = []
    for k in range(NCORES):
        lo, hi = k * DREAL, (k + 1) * DREAL
        wTk = np.zeros((F, DP), dtype=np.float32)
        wTk[:, :DREAL] = bhv_matrix[lo:hi, :].T
        cTk = np.full((DP, C), 0.5, dtype=np.float32)
        cTk[:DREAL, :] = centroids[:, lo:hi].T
        in_maps.append({"xT": xT, "wT": wTk, "cT": cTk})
    return in_maps


def _run(samples, bhv_matrix, centroids, **spmd_kwargs):
    nc = _build_nc()
    in_maps = _prep_in_maps(samples, bhv_matrix, centroids)
    res = run_bass_kernel_spmd(nc, in_maps, core_ids=list(range(NCORES)),
                               **spmd_kwargs)
    acc = np.zeros((C, B), dtype=np.float32)
    for r in res.results:
        acc += r["out"]
    return np.ascontiguousarray(acc.T), res


def kernel(samples, bhv_matrix, centroids):
    out, _ = _run(samples, bhv_matrix, centroids)
    return out
